# revision 1
# baseline (speedup 1.0000x reference)
"""CrossAttn + TISA bias kernel for TRN2, 8-core SPMD.

Sharding: core = (batch b = core//2, query half = core%2).
Each core computes the full kv projection for its batch (duplicated within
the pair) and its 512 query rows end-to-end. No collectives.

Inputs arrive host-transposed: xqt/xkvt are [d_in, tokens].
  qT:   [d_out(part), i]   (scaled by 1/sqrt(Dh))
  kT:   [d_out(part), j]
  v:    [j(part), d_out]
  S^T:  [j(part), i] = kT_h.T @ qT_h          (K=64)
  wT = exp(S^T) * srow[:, C:C+512]            (shifted exp-bias table slice)
  sums: mask-matmul E2.T @ wT -> psum [2, 512]
  attn: v_h.T @ wT -> psum chunk rows {0:64, 64:128} per head pair
  attn_norm = attn * bcast(1/sums)            (PE broadcast of recip)
  gate: attn_norm.T @ Wg -> [i(part), 2048]; out = (a+bga)*sigmoid(b+bgb)
"""

import numpy as np
import ml_dtypes

import concourse.bacc as bacc
import concourse.mybir as mybir
import concourse.tile as tile
from concourse.bass import ts

L = 1024
D = 1024
H = 16
DH = 64
LQ = 512          # q rows per core
NIC = LQ // 128   # 4 i-chunks
NJC = L // 128    # 8 j-chunks
NKC = D // 128    # 8 d_model chunks
SROW_W = 1408
NUM_KERNELS = 21

F32 = mybir.dt.float32
EXP = mybir.ActivationFunctionType.Exp
SIG = mybir.ActivationFunctionType.Sigmoid
CPY = mybir.ActivationFunctionType.Copy
MUL = mybir.AluOpType.mult
ADD = mybir.AluOpType.add

_DT = {"f32": mybir.dt.float32, "bf16": mybir.dt.bfloat16}
_NP = {"f32": np.float32, "bf16": ml_dtypes.bfloat16}


def ds2(hh):
    return slice(hh * 64, hh * 64 + 64)


def build_nc(cfg="bf16"):
    mdt = _DT[cfg]
    sdt = mdt   # srow/wexp dtype rides the matmul dtype
    resident = cfg == "bf16"

    nc = bacc.Bacc("TRN2", target_bir_lowering=False, debug=False, num_devices=8)

    xqt_d = nc.dram_tensor("xqt", [D, LQ], mdt, kind="ExternalInput").ap()
    xkvt_d = nc.dram_tensor("xkvt", [D, L], mdt, kind="ExternalInput").ap()
    wq_d = nc.dram_tensor("wq", [D, D], mdt, kind="ExternalInput").ap()
    wm_d = nc.dram_tensor("wm", [D, 2 * D], mdt, kind="ExternalInput").ap()
    wg_d = nc.dram_tensor("wg", [D, 2 * D], mdt, kind="ExternalInput").ap()
    srow_d = nc.dram_tensor("srow", [H, 128, SROW_W], sdt, kind="ExternalInput").ap()
    bg_d = nc.dram_tensor("bgrep", [128, 2 * D], F32, kind="ExternalInput").ap()
    e2_d = nc.dram_tensor("e2", [128, 4], mdt, kind="ExternalInput").ap()
    p2_d = nc.dram_tensor("p2", [2, 128], F32, kind="ExternalInput").ap()
    out_d = nc.dram_tensor("out", [LQ, D], F32, kind="ExternalOutput").ap()

    with tile.TileContext(nc) as tc:
        with (
            tc.tile_pool(name="const", bufs=1) as constp,
            tc.tile_pool(name="persist", bufs=1) as pers,
            tc.tile_pool(name="psum", bufs=1, space="PSUM") as psum,
            tc.tile_pool(name="phB", bufs=1) as phb,
            tc.tile_pool(name="phC", bufs=1) as phc,
            tc.tile_pool(name="phD", bufs=1) as phd,
        ):
            e2_sb = constp.tile([128, 4], mdt)
            nc.sync.dma_start(out=e2_sb, in_=e2_d)
            p2_sb = constp.tile([2, 128], F32)
            nc.sync.dma_start(out=p2_sb, in_=p2_d)
            bg_sb = constp.tile([128, 2 * D], F32)
            nc.sync.dma_start(out=bg_sb, in_=bg_d)

            qT = pers.tile([128, NKC, LQ], mdt)        # [d_out, mc, i]
            kT = pers.tile([128, NKC, L], mdt)         # [d_out, mc, j]
            vsb = pers.tile([128, NJC, D], mdt)        # [j, jc, d_out]
            attn = pers.tile([128, NKC, LQ], mdt)      # [d_model, chunk, i]

            # =========== phase B: projections ==========
            if True:
                xqT = phb.tile([128, NKC, LQ], mdt)    # [d_in, kc, i]
                xkvT = phb.tile([128, NKC, L], mdt)    # [d_in, kc, j]
                for kc in range(NKC):
                    nc.sync.dma_start(out=xqT[:, kc, :], in_=xqt_d[ts(kc, 128), :])
                    nc.sync.dma_start(out=xkvT[:, kc, :], in_=xkvt_d[ts(kc, 128), :])

                if resident:
                    wq_r = phb.tile([128, NKC, D], mdt)
                    wm_r = phb.tile([128, NKC, 2 * D], mdt)
                    for kc in range(NKC):
                        nc.sync.dma_start(out=wq_r[:, kc, :], in_=wq_d[ts(kc, 128), :])
                        nc.sync.dma_start(out=wm_r[:, kc, :], in_=wm_d[ts(kc, 128), :])

                def get_wq(kc, cols):
                    if resident:
                        return wq_r[:, kc, cols]
                    t = phb.tile([128, 128], mdt, tag="wqs", bufs=3)
                    nc.sync.dma_start(out=t, in_=wq_d[ts(kc, 128), cols])
                    return t

                def get_wm(kc, cols, n):
                    if resident:
                        return wm_r[:, kc, cols]
                    t = phb.tile([128, n], mdt, tag=f"wms{n}", bufs=3)
                    nc.sync.dma_start(out=t, in_=wm_d[ts(kc, 128), cols])
                    return t

                for mc in range(NKC):
                    ps = psum.tile([128, LQ], F32, tag="t1", bufs=4)
                    for kc in range(NKC):
                        nc.tensor.matmul(
                            ps, get_wq(kc, ts(mc, 128)), xqT[:, kc, :],
                            start=(kc == 0), stop=(kc == NKC - 1))
                    nc.scalar.activation(qT[:, mc, :], ps, CPY, scale=0.125)

                for mc in range(NKC):
                    for nh in range(2):
                        ps = psum.tile([128, 512], F32, tag="t1", bufs=4)
                        for kc in range(NKC):
                            nc.tensor.matmul(
                                ps, get_wm(kc, ts(mc, 128), 128),
                                xkvT[:, kc, ts(nh, 512)],
                                start=(kc == 0), stop=(kc == NKC - 1))
                        if nh == 0:
                            nc.vector.tensor_copy(kT[:, mc, ts(nh, 512)], ps)
                        else:
                            nc.scalar.activation(kT[:, mc, ts(nh, 512)], ps, CPY)

                for jc in range(NJC):
                    for nh in range(2):
                        ps = psum.tile([128, 512], F32, tag="t1", bufs=4)
                        for kc in range(NKC):
                            nc.tensor.matmul(
                                ps, xkvT[:, kc, ts(jc, 128)],
                                get_wm(kc, slice(D + nh * 512, D + nh * 512 + 512), 512),
                                start=(kc == 0), stop=(kc == NKC - 1))
                        if nh == 0:
                            nc.vector.tensor_copy(vsb[:, jc, ts(nh, 512)], ps)
                        else:
                            nc.scalar.activation(vsb[:, jc, ts(nh, 512)], ps, CPY)

            # ================= phase C: attention =================
            if True:
                for c in range(NKC):
                    ps_at = psum.tile([128, LQ], F32, tag="attn", bufs=2)
                    ps_sum = psum.tile([2, LQ], F32, tag="sums", bufs=2)
                    for hh in range(2):
                        h = 2 * c + hh
                        srow_sb = phc.tile([128, SROW_W], sdt, tag="srow", bufs=2)
                        nc.sync.dma_start(out=srow_sb, in_=srow_d[h, :, :])
                        wts = []
                        for jc in range(NJC):
                            ps_s = psum.tile([128, LQ], F32, tag="t1", bufs=4)
                            nc.tensor.matmul(
                                ps_s, kT[ds2(hh), c, ts(jc, 128)], qT[ds2(hh), c, :],
                                start=True, stop=True)
                            wexp = phc.tile([128, LQ], sdt, tag="wexp", bufs=4)
                            nc.scalar.activation(wexp, ps_s, EXP)
                            wT = phc.tile([128, LQ], mdt, tag="wt", bufs=8)
                            C0 = 896 - jc * 128
                            nc.vector.tensor_tensor(
                                wT, wexp, srow_sb[:, C0:C0 + LQ], MUL)
                            wts.append(wT)
                        for jc in range(NJC):
                            nc.tensor.matmul(
                                ps_sum, e2_sb[:, 2 * hh:2 * hh + 2], wts[jc],
                                start=(hh == 0 and jc == 0),
                                stop=(hh == 1 and jc == NJC - 1))
                            nc.tensor.matmul(
                                ps_at[ds2(hh), :], vsb[:, jc, ts(h, DH)], wts[jc],
                                start=(jc == 0), stop=(jc == NJC - 1))
                    rsum = phc.tile([2, LQ], F32, tag="rsum", bufs=2)
                    nc.vector.reciprocal(rsum, ps_sum)
                    ps_rb = psum.tile([128, LQ], F32, tag="t1", bufs=4)
                    nc.tensor.matmul(ps_rb, p2_sb, rsum, start=True, stop=True)
                    rb = phc.tile([128, LQ], F32, tag="rb", bufs=2)
                    nc.scalar.activation(rb, ps_rb, CPY)
                    nc.vector.tensor_tensor(attn[:, c, :], ps_at, rb, MUL)

            # ================= phase D: gate =================
            if True:
                if resident:
                    wg_r = phd.tile([128, NKC, 2 * D], mdt)
                    for kc in range(NKC):
                        nc.sync.dma_start(out=wg_r[:, kc, :], in_=wg_d[ts(kc, 128), :])

                def get_wg(kc, cols):
                    if resident:
                        return wg_r[:, kc, cols]
                    t = phd.tile([128, 512], mdt, tag="wgs", bufs=3)
                    nc.sync.dma_start(out=t, in_=wg_d[ts(kc, 128), cols])
                    return t

                for ic in range(NIC):
                    out_t = phd.tile([128, D], F32, tag="outt", bufs=2)
                    for qa in range(2):
                        ps_a = psum.tile([128, 512], F32, tag="t1", bufs=4)
                        ps_b = psum.tile([128, 512], F32, tag="t1", bufs=4)
                        for kc in range(NKC):
                            nc.tensor.matmul(
                                ps_a, attn[:, kc, ts(ic, 128)], get_wg(kc, ts(qa, 512)),
                                start=(kc == 0), stop=(kc == NKC - 1))
                        for kc in range(NKC):
                            nc.tensor.matmul(
                                ps_b, attn[:, kc, ts(ic, 128)],
                                get_wg(kc, slice(D + qa * 512, D + qa * 512 + 512)),
                                start=(kc == 0), stop=(kc == NKC - 1))
                        ta = phd.tile([128, 512], F32, tag="ta", bufs=2)
                        nc.vector.tensor_tensor(ta, ps_a, bg_sb[:, ts(qa, 512)], ADD)
                        tb = phd.tile([128, 512], F32, tag="tb", bufs=2)
                        nc.vector.tensor_tensor(
                            tb, ps_b, bg_sb[:, D + qa * 512:D + qa * 512 + 512], ADD)
                        tsg = phd.tile([128, 512], F32, tag="tsg", bufs=2)
                        nc.scalar.activation(tsg, tb, SIG)
                        nc.vector.tensor_tensor(out_t[:, ts(qa, 512)], ta, tsg, MUL)
                    nc.sync.dma_start(out=out_d[ts(ic, 128), :], in_=out_t)

    nc.compile()
    return nc


# ======================= host side =======================

def _tisa_ebias(amp, off, sharp):
    d = np.arange(-(L - 1), L, dtype=np.float32)
    s = np.sum(
        amp[:, :, None].astype(np.float32)
        * np.exp(-np.abs(sharp)[:, :, None].astype(np.float32)
                 * (d[None, None, :] - off[:, :, None].astype(np.float32)) ** 2),
        axis=1, dtype=np.float32).astype(np.float32)
    return np.exp(s).astype(np.float32)


def make_host_inputs(inputs, cfg="bf16"):
    npdt = _NP[cfg]
    x_q = np.asarray(inputs["x_q"])
    x_kv = np.asarray(inputs["x_kv"])
    wq = np.asarray(inputs["Wq"]).astype(npdt)
    wm = np.asarray(inputs["Wm"]).astype(npdt)
    wg = np.asarray(inputs["Wg"]).astype(npdt)
    bg = np.asarray(inputs["bg"]).astype(np.float32)

    ebias = _tisa_ebias(np.asarray(inputs["tisa_amp"]),
                        np.asarray(inputs["tisa_off"]),
                        np.asarray(inputs["tisa_sharp"]))

    p_i = np.arange(128)[:, None]
    m_i = np.arange(SROW_W)[None, :]
    srows = []
    for i_off in (0, 512):
        idx = p_i - m_i + (1919 - i_off)
        srows.append(np.ascontiguousarray(ebias[:, idx]).astype(npdt))

    e2 = np.zeros((128, 4), dtype=npdt)
    e2[:, 0] = 1
    e2[:, 3] = 1
    p2 = np.zeros((2, 128), dtype=np.float32)
    p2[0, :64] = 1
    p2[1, 64:] = 1
    bgrep = np.ascontiguousarray(np.broadcast_to(bg, (128, 2 * D))).astype(np.float32)

    in_maps = []
    for core in range(8):
        b, half = core // 2, core % 2
        in_maps.append({
            "xqt": np.ascontiguousarray(
                x_q[b, half * LQ:(half + 1) * LQ].T).astype(npdt),
            "xkvt": np.ascontiguousarray(x_kv[b].T).astype(npdt),
            "wq": wq, "wm": wm, "wg": wg,
            "srow": srows[half],
            "bgrep": bgrep, "e2": e2, "p2": p2,
        })
    return in_maps


def assemble_output(results):
    out = np.empty((4, L, D), dtype=np.float32)
    for core in range(8):
        b, half = core // 2, core % 2
        out[b, half * LQ:(half + 1) * LQ] = results[core]["out"]
    return out


# ======================= public entry point =======================

_NC_CACHE = {}


def _get_nc(cfg):
    if cfg not in _NC_CACHE:
        _NC_CACHE[cfg] = build_nc(cfg)
    return _NC_CACHE[cfg]


def kernel(**inputs):
    """Full (unsharded) inputs -> full (4, 1024, 1024) float32 output.

    Shards over 8 NeuronCores: core = (batch, query-half). Host precomputes
    the TISA exp-bias lookup table and pre-transposes activations; all dense
    compute (projections, attention, gate) runs on-device in bf16 matmuls
    with fp32 accumulation.
    """
    from concourse.bass_utils import run_bass_kernel_spmd

    cfg = "bf16"
    nc = _get_nc(cfg)
    in_maps = make_host_inputs(inputs, cfg)
    res = run_bass_kernel_spmd(nc, in_maps, core_ids=list(range(8)))
    return assemble_output(res.results)



# revision 16
# speedup vs baseline: 1.2318x; 1.2318x over previous
"""CrossAttn + TISA bias kernel for TRN2, 8-core SPMD.

Sharding: core = (batch b = core//2, query half = core%2).
Each core computes the full kv projection for its batch (duplicated within
the pair) and its 512 query rows end-to-end. No collectives.

Inputs arrive host-transposed: xqt/xkvt are [d_in, tokens].
  qT:   [d_out(part), i]   (scaled by 1/sqrt(Dh))
  kT:   [d_out(part), j]
  vsb:  [j(part), jc, head, 65]  with a ones column at slot 64 per head
  S^T:  [j(part), i] = kT_h.T @ qT_h          (K=64), pairs of j-chunks
        land in one [128,2,512] PSUM tile so each Exp covers 1024 elems
  wT = exp(S^T) * srow[:, C:C+512]            (shifted exp-bias table slice)
  attn+sums: v_h(65 cols).T @ wT -> psum [65, 512]; row 64 = softmax denom
  norm: recip (DVE) -> p2 broadcast matmul (f32r) -> rb -> psum*rb,
        software-pipelined one chunk behind attention
  gate: attn_norm.T @ Wg -> [i(part), 2048]; out = (a+bga)*sigmoid(b+bgb)

Phase B runs kc-outer "waves" of 8 open PSUM accumulation groups so matmuls
consume weight chunks in DMA arrival order.
"""

import numpy as np
import ml_dtypes

import concourse.bacc as bacc
import concourse.mybir as mybir
import concourse.tile as tile
from concourse.bass import ts

L = 1024
D = 1024
H = 16
DH = 64
LQ = 512          # q rows per core
NIC = LQ // 128   # 4 i-chunks
NJC = L // 128    # 8 j-chunks
NKC = D // 128    # 8 d_model chunks
SROW_W = 1408
NUM_KERNELS = 21

F32 = mybir.dt.float32
F32R = mybir.dt.float32r
EXP = mybir.ActivationFunctionType.Exp
SIG = mybir.ActivationFunctionType.Sigmoid
CPY = mybir.ActivationFunctionType.Copy
MUL = mybir.AluOpType.mult
ADD = mybir.AluOpType.add

_DT = {"f32": mybir.dt.float32, "bf16": mybir.dt.bfloat16}
_NP = {"f32": np.float32, "bf16": ml_dtypes.bfloat16}


def ds2(hh):
    return slice(hh * 64, hh * 64 + 64)


def build_nc(cfg="bf16"):
    mdt = _DT[cfg]
    sdt = mdt

    nc = bacc.Bacc("TRN2", target_bir_lowering=False, debug=False, num_devices=8)

    xqt_d = nc.dram_tensor("xqt", [D, LQ], mdt, kind="ExternalInput").ap()
    xkvt_d = nc.dram_tensor("xkvt", [D, L], mdt, kind="ExternalInput").ap()
    wq_d = nc.dram_tensor("wq", [D, D], mdt, kind="ExternalInput").ap()
    wm_d = nc.dram_tensor("wm", [D, 2 * D], mdt, kind="ExternalInput").ap()
    wg_d = nc.dram_tensor("wg", [D, 2 * D], mdt, kind="ExternalInput").ap()
    srow_d = nc.dram_tensor("srow", [H, 128, SROW_W], sdt, kind="ExternalInput").ap()
    bg_d = nc.dram_tensor("bgrep", [128, 2 * D], mdt, kind="ExternalInput").ap()
    out_d = nc.dram_tensor("out", [LQ, D], F32, kind="ExternalOutput").ap()

    with tile.TileContext(nc) as tc:
        with (
            tc.tile_pool(name="const", bufs=1) as constp,
            tc.tile_pool(name="persist", bufs=1) as pers,
            tc.tile_pool(name="psum", bufs=1, space="PSUM") as psum,
            tc.tile_pool(name="phB", bufs=1) as phb,
            tc.tile_pool(name="phC", bufs=1) as phc,
            tc.tile_pool(name="phD", bufs=1) as phd,
        ):
            # resident inputs (DMA issue order = emission order on SP queue;
            # transfers serialize on the DMA engines in this order)
            xqT = phb.tile([128, NKC, LQ], mdt)    # [d_in, kc, i]
            wq_r = phb.tile([128, NKC, D], mdt)
            for kc in range(NKC):
                nc.sync.dma_start(out=wq_r[:, kc, :], in_=wq_d[ts(kc, 128), :])
                nc.sync.dma_start(out=xqT[:, kc, :], in_=xqt_d[ts(kc, 128), :])

            xkvT = phb.tile([128, NKC, L], mdt)    # [d_in, kc, j]
            wm_r = phb.tile([128, NKC, 2 * D], mdt)
            for kc in range(NKC):
                nc.sync.dma_start(out=wm_r[:, kc, :], in_=wm_d[ts(kc, 128), :])
                nc.sync.dma_start(out=xkvT[:, kc, :], in_=xkvt_d[ts(kc, 128), :])

            e1_sb = constp.tile([1, 128], mdt)
            nc.vector.memset(e1_sb, 1.0)

            def load_srow(h):
                t = phc.tile([128, SROW_W], sdt, tag="srow", bufs=3, name="srow")
                nc.sync.dma_start(out=t, in_=srow_d[h, :, :])
                return t

            srow_tiles = {h: load_srow(h) for h in (0, 1, 2, 3)}

            wg_r = phd.tile([128, NKC, 2 * D], mdt)
            for kc in range(NKC):
                nc.sync.dma_start(out=wg_r[:, kc, :], in_=wg_d[ts(kc, 128), :])
            bg_sb = constp.tile([128, 2 * D], mdt)
            nc.sync.dma_start(out=bg_sb, in_=bg_d)

            qT = pers.tile([128, NKC, LQ], mdt)        # [d_out, mc, i]
            kT = pers.tile([128, NKC, L], mdt)         # [d_out, mc, j]
            vsb = pers.tile([128, NJC, H, 65], mdt)    # [j, jc, head, v+ones]
            attn = pers.tile([128, NKC, LQ], mdt)      # [d_model, chunk, i]

            for jc in range(NJC):
                nc.vector.memset(vsb[:, jc, :, 64:65], 1.0)

            def sc_tile():
                return psum.tile([128, 2, 512], F32, tag="sc", bufs=2, name="sc")

            def at_tile():
                return psum.tile([128, LQ], F32, tag="at", bufs=4, name="at")

            def wave_groups():
                ats = [at_tile() for _ in range(4)]
                scs = [sc_tile() for _ in range(2)]
                return [ats[0], ats[1], ats[2], ats[3],
                        scs[0][:, 0, :], scs[0][:, 1, :],
                        scs[1][:, 0, :], scs[1][:, 1, :]]

            def drain(dst, src, idx, scale=None):
                # alternate engines so bank-release isn't single-queue bound
                if scale is not None:
                    if idx % 2 == 0:
                        nc.scalar.activation(dst, src, CPY, scale=scale)
                    else:
                        nc.vector.tensor_scalar_mul(dst, src, scale)
                elif idx % 2 == 0:
                    nc.vector.tensor_copy(dst, src)
                else:
                    nc.scalar.activation(dst, src, CPY)

            # softmax normalization, software-pipelined one section behind
            pend = [None]  # (ps0, ps1, r2, c) awaiting normalization

            def flush_norm():
                if pend[0] is None:
                    return
                ps0, ps1, r2a, r2b, pc = pend[0]
                pend[0] = None
                ps_rb = sc_tile()
                nc.tensor.matmul(ps_rb[:, 0, :][0:64, :], e1_sb[:, 0:64], r2a,
                                 start=True, stop=True)
                nc.tensor.matmul(ps_rb[:, 0, :][64:128, :], e1_sb[:, 64:128],
                                 r2b, start=True, stop=True)
                rb = phc.tile([128, LQ], F32, tag="rb", bufs=2)
                nc.vector.tensor_copy(rb, ps_rb[:, 0, :])
                nc.vector.tensor_tensor(attn[ds2(0), pc, :], ps0[0:64, :],
                                        rb[ds2(0), :], MUL)
                nc.vector.tensor_tensor(attn[ds2(1), pc, :], ps1[0:64, :],
                                        rb[ds2(1), :], MUL)

            # ======== phase B: projection waves (kc-outer, 8 groups) ========
            def q_wave():
                grps = wave_groups()
                for kc in range(NKC):
                    for mc in range(NKC):
                        nc.tensor.matmul(
                            grps[mc], wq_r[:, kc, ts(mc, 128)], xqT[:, kc, :],
                            start=(kc == 0), stop=(kc == NKC - 1))
                for mc in range(NKC):
                    drain(qT[:, mc, :], grps[mc], mc, scale=0.125)

            # ("k", mc, jh) -> kT[:, mc, jh*512:], contract kc
            # ("v", jc, nh) -> vsb[:, jc, nh*8:(nh+1)*8, 0:64]
            def kv_wave(wave):
                flush_norm()
                grps = wave_groups()
                for kc in range(NKC):
                    for g, (kind, a, b) in enumerate(wave):
                        if kind == "k":
                            lhsT = wm_r[:, kc, ts(a, 128)]
                            rhs = xkvT[:, kc, ts(b, 512)]
                        else:
                            lhsT = xkvT[:, kc, ts(a, 128)]
                            rhs = wm_r[:, kc, slice(D + b * 512, D + b * 512 + 512)]
                        nc.tensor.matmul(grps[g], lhsT, rhs,
                                         start=(kc == 0), stop=(kc == NKC - 1))
                for g, (kind, a, b) in enumerate(wave):
                    if kind == "k":
                        dst = kT[:, a, ts(b, 512)]
                    else:
                        dst = vsb[:, a, b * 8:(b + 1) * 8, 0:64]
                    drain(dst, grps[g], g)

            # ============ phase C: attention for one head-pair chunk ========
            srow_next = [4]

            def attn_chunk(c):
                while srow_next[0] < min(2 * c + 8, H):
                    h = srow_next[0]
                    srow_tiles[h] = load_srow(h)
                    srow_next[0] += 1
                ps_h = [None, None]
                r2s = [None, None]
                for hh in range(2):
                    h = 2 * c + hh
                    srow_sb = srow_tiles.pop(h)
                    ps_at = at_tile()
                    ps_h[hh] = ps_at
                    # bias multiplies: most on DVE, a few on the idle GpSimd.
                    # attn matmuls accumulate in any order, so DVE-produced
                    # weights go first and slow GpSimd ones last.
                    pool_jc = ()
                    wts = {}
                    for t in range(4):
                        sct = sc_tile()
                        for u in range(2):
                            jc = 2 * t + u
                            nc.tensor.matmul(
                                sct[:, u, :],
                                kT[ds2(hh), c, ts(jc, 128)], qT[ds2(hh), c, :],
                                start=True, stop=True)
                        if hh == 0 and t == 1:
                            flush_norm()
                        wexp = phc.tile([128, 2, LQ], sdt, tag="wexp", bufs=3)
                        nc.scalar.activation(wexp, sct, EXP)
                        for u in range(2):
                            jc = 2 * t + u
                            wT = phc.tile([128, LQ], mdt, tag="wt", bufs=8)
                            C0 = 896 - jc * 128
                            eng = nc.gpsimd if jc in pool_jc else nc.vector
                            eng.tensor_tensor(
                                wT, wexp[:, u, :], srow_sb[:, C0:C0 + LQ], MUL)
                            wts[jc] = wT
                        for jc in range(2 * t + 2):
                            if jc in wts and jc not in pool_jc:
                                nc.tensor.matmul(
                                    ps_at[0:65, :], vsb[:, jc, h, :],
                                    wts.pop(jc), start=(t == 0 and jc == 0),
                                    stop=False)
                    for n, jc in enumerate(sorted(wts)):
                        nc.tensor.matmul(
                            ps_at[0:65, :], vsb[:, jc, h, :], wts[jc],
                            start=False, stop=(n == len(wts) - 1))
                    r2h = phc.tile([1, LQ], mdt, tag="r2", bufs=4, name="r2")
                    with nc.allow_low_precision(
                            reason="softmax denom recip in bf16 is plenty"):
                        nc.vector.reciprocal(r2h, ps_at[64:65, :])
                    r2s[hh] = r2h
                pend[0] = (ps_h[0], ps_h[1], r2s[0], r2s[1], c)

            # interleave attention chunks between projection waves so the
            # Activation engine (exp-bound) starts working during the
            # PE-bound projection phase
            q_wave()
            kv_wave([("k", 0, 0), ("k", 0, 1), ("k", 1, 0), ("k", 1, 1),
                     ("v", 0, 0), ("v", 1, 0), ("v", 2, 0), ("v", 3, 0)])
            kv_wave([("k", 2, 0), ("k", 2, 1), ("k", 3, 0), ("k", 3, 1),
                     ("v", 4, 0), ("v", 5, 0), ("v", 6, 0), ("v", 7, 0)])
            kv_wave([("k", 4, 0), ("k", 4, 1), ("k", 5, 0), ("k", 5, 1),
                     ("v", 0, 1), ("v", 1, 1), ("v", 2, 1), ("v", 3, 1)])
            kv_wave([("k", 6, 0), ("k", 6, 1), ("k", 7, 0), ("k", 7, 1),
                     ("v", 4, 1), ("v", 5, 1), ("v", 6, 1), ("v", 7, 1)])
            for c in range(NKC):
                attn_chunk(c)

            # ================= phase D: gate =================
            # warm-start: accumulate the first gate group over chunks 0..6
            # while chunk 7's normalization drains, then finish with kc=7.
            for ic in range(NIC):
                out_t = phd.tile([128, D], F32, tag="outt", bufs=2)
                for qa in range(2):
                    ps_b = at_tile()
                    for kc in range(NKC - 1):
                        nc.tensor.matmul(
                            ps_b, attn[:, kc, ts(ic, 128)],
                            wg_r[:, kc, slice(D + qa * 512, D + qa * 512 + 512)],
                            start=(kc == 0), stop=False)
                    flush_norm()
                    nc.tensor.matmul(
                        ps_b, attn[:, NKC - 1, ts(ic, 128)],
                        wg_r[:, NKC - 1, slice(D + qa * 512, D + qa * 512 + 512)],
                        start=False, stop=True)
                    ps_a = at_tile()
                    for kc in range(NKC):
                        nc.tensor.matmul(
                            ps_a, attn[:, kc, ts(ic, 128)],
                            wg_r[:, kc, ts(qa, 512)],
                            start=(kc == 0), stop=(kc == NKC - 1))
                    tb = phd.tile([128, 512], F32, tag="tb", bufs=2)
                    nc.vector.tensor_tensor(
                        tb, ps_b, bg_sb[:, D + qa * 512:D + qa * 512 + 512], ADD)
                    tsg = phd.tile([128, 512], F32, tag="tsg", bufs=2)
                    nc.scalar.activation(tsg, tb, SIG)
                    last = (ic == NIC - 1 and qa == 1)
                    # final chunk: 2 pipelined sub-chains shorten the tail
                    subs = ([(0, 256), (256, 256)] if last
                            else [(0, 512)])
                    ta = phd.tile([128, 512], F32, tag="ta", bufs=2)
                    for s0, sw in subs:
                        sl = slice(s0, s0 + sw)
                        osl = slice(qa * 512 + s0, qa * 512 + s0 + sw)
                        nc.vector.tensor_tensor(
                            ta[:, sl], ps_a[:, sl],
                            bg_sb[:, qa * 512 + s0:qa * 512 + s0 + sw], ADD)
                        nc.vector.tensor_tensor(
                            out_t[:, osl], ta[:, sl], tsg[:, sl], MUL)
                        nc.sync.dma_start(
                            out=out_d[ts(ic, 128), osl], in_=out_t[:, osl])

    nc.compile()
    return nc


# ======================= host side =======================

def _tisa_ebias(amp, off, sharp):
    d = np.arange(-(L - 1), L, dtype=np.float32)
    s = np.sum(
        amp[:, :, None].astype(np.float32)
        * np.exp(-np.abs(sharp)[:, :, None].astype(np.float32)
                 * (d[None, None, :] - off[:, :, None].astype(np.float32)) ** 2),
        axis=1, dtype=np.float32).astype(np.float32)
    return np.exp(s).astype(np.float32)


def make_host_inputs(inputs, cfg="bf16"):
    npdt = _NP[cfg]
    x_q = np.asarray(inputs["x_q"])
    x_kv = np.asarray(inputs["x_kv"])
    wq = np.asarray(inputs["Wq"]).astype(npdt)
    wm = np.asarray(inputs["Wm"]).astype(npdt)
    wg = np.asarray(inputs["Wg"]).astype(npdt)
    bg = np.asarray(inputs["bg"]).astype(np.float32)

    ebias = _tisa_ebias(np.asarray(inputs["tisa_amp"]),
                        np.asarray(inputs["tisa_off"]),
                        np.asarray(inputs["tisa_sharp"]))

    p_i = np.arange(128)[:, None]
    m_i = np.arange(SROW_W)[None, :]
    srows = []
    for i_off in (0, 512):
        idx = p_i - m_i + (1919 - i_off)
        srows.append(np.ascontiguousarray(ebias[:, idx]).astype(npdt))

    bgrep = np.ascontiguousarray(np.broadcast_to(bg, (128, 2 * D))).astype(npdt)

    in_maps = []
    for core in range(8):
        b, half = core // 2, core % 2
        in_maps.append({
            "xqt": np.ascontiguousarray(
                x_q[b, half * LQ:(half + 1) * LQ].T).astype(npdt),
            "xkvt": np.ascontiguousarray(x_kv[b].T).astype(npdt),
            "wq": wq, "wm": wm, "wg": wg,
            "srow": srows[half],
            "bgrep": bgrep,
        })
    return in_maps


def assemble_output(results):
    out = np.empty((4, L, D), dtype=np.float32)
    for core in range(8):
        b, half = core // 2, core % 2
        out[b, half * LQ:(half + 1) * LQ] = results[core]["out"]
    return out


# ======================= public entry point =======================

_NC_CACHE = {}


def _get_nc(cfg):
    if cfg not in _NC_CACHE:
        _NC_CACHE[cfg] = build_nc(cfg)
    return _NC_CACHE[cfg]


def kernel(**inputs):
    """Full (unsharded) inputs -> full (4, 1024, 1024) float32 output.

    Shards over 8 NeuronCores: core = (batch, query-half). Host precomputes
    the TISA exp-bias lookup table and pre-transposes activations; all dense
    compute (projections, attention, gate) runs on-device in bf16 matmuls
    with fp32 accumulation.
    """
    from concourse.bass_utils import run_bass_kernel_spmd

    cfg = "bf16"
    nc = _get_nc(cfg)
    in_maps = make_host_inputs(inputs, cfg)
    res = run_bass_kernel_spmd(nc, in_maps, core_ids=list(range(8)))
    return assemble_output(res.results)
